# revision 2
# baseline (speedup 1.0000x reference)
# ContextViT v57 Trainium2 Bass kernel.
# Strategy: data-parallel over batch (4 images per NeuronCore x 8 cores), no collectives.
# Activations live feature-major in SBUF ([768, tokens]) for the whole 12-block stack;
# weights stream from HBM. Matmuls run as fp32r (1 cyc/row at moving dim >= 256).
import os
import numpy as np

import concourse.tile as tile
from concourse import bacc, mybir
from concourse.bass_utils import run_bass_kernel_spmd

F32 = mybir.dt.float32
F32R = mybir.dt.float32r
BF16 = mybir.dt.bfloat16
AF = mybir.ActivationFunctionType
ALU = mybir.AluOpType
AX = mybir.AxisListType

DEPTH = int(os.environ.get("VIT_DEPTH", "12"))
D = 768
H = 12
DH = 64
GRID = 14
PH = 16
P = GRID * GRID            # 196 patches
SN = P + 1                 # 197 tokens per sample
TD = 2
T = P // (TD * TD)         # 49 tiles
K = 1 + T                  # 50 ctx tokens
HID = 4 * D
CPB_HID = 32
NCORES = 8
BTOT = 32
B = BTOT // NCORES         # 4 samples per core
TOK = B * SN               # 788 columns
DC = D // 128              # 6 feature chunks
HC = HID // 128            # 24 hidden chunks
HALF = TOK // 2            # 394
CTXP = 64                  # padded ctx tokens per sample
CTOK = B * CTXP            # 256
PTOK = B * P               # 784 patch cols
PHALF = PTOK // 2          # 392
VW = 65                    # v cols per head (64 + ones col)
NK = SN * K                # 9850 cpb positions
CPC = 256                  # cpb col chunk
EPS = 1e-5


def _tile_major_perm():
    perm = []
    for ti in range(GRID // TD):
        for tj in range(GRID // TD):
            for a in range(TD):
                for b in range(TD):
                    perm.append((TD * ti + a) * GRID + (TD * tj + b))
    return np.array(perm, dtype=np.int64)


def prep_inputs(inp):
    """Host-side layout prep + algebraic weight folds. Returns (shared dict, per-core list)."""
    perm = _tile_major_perm()
    f = lambda a: np.asarray(a, dtype=np.float32)
    C = np.ascontiguousarray

    n1w = f(inp["n1_w"]); n1b = f(inp["n1_b"])
    n2w = f(inp["n2_w"]); n2b = f(inp["n2_b"])
    qw = f(inp["q_w"]); kvw = f(inp["kv_w"])
    ow = f(inp["out_w"]); ob = f(inp["out_b"])
    lw = f(inp["logit_w"])
    f1w = f(inp["fc1_w"]); f1b = f(inp["fc1_b"])
    f2w = f(inp["fc2_w"]); f2b = f(inp["fc2_b"])

    L = DEPTH
    sc = 1.0 / np.sqrt(DH)
    qwT = np.stack([C((qw[l] * n1w[l][None, :] * sc).T) for l in range(L)])
    qb = np.stack([(qw[l] @ n1b[l] * sc)[:, None] for l in range(L)])
    kwT = np.stack([C((kvw[l][:D] * n1w[l][None, :]).T) for l in range(L)])
    vwT = np.stack([C((kvw[l][D:] * n1w[l][None, :]).T) for l in range(L)])
    lwT = np.stack([(lw[l][0] * n1w[l])[:, None] for l in range(L)])
    owT = np.stack([C(ow[l].T) for l in range(L)])
    obf = np.stack([(ob[l] + ow[l] @ (kvw[l][D:] @ n1b[l]))[:, None] for l in range(L)])
    f1wT = np.stack([C((f1w[l] * n2w[l][None, :]).T) for l in range(L)])
    f1bf = np.stack([(f1b[l] + f1w[l] @ n2b[l])[:, None] for l in range(L)])
    f2wT = np.stack([C(f2w[l].T) for l in range(L)])
    f2bf = np.stack([f2b[l][:, None] for l in range(L)])

    w1T = np.stack([C(f(inp["cpb_w1"])[l].T) for l in range(L)])
    b1 = np.stack([f(inp["cpb_b1"])[l][:, None] for l in range(L)])
    w2T = np.stack([C(f(inp["cpb_w2"])[l].T) for l in range(L)])
    b2 = np.stack([f(inp["cpb_b2"])[l][:, None] for l in range(L)])

    feats = f(inp["cpb_feats"])
    mask = f(inp["cpb_mask"])
    tokperm = np.concatenate([[0], perm + 1])
    featsT = C(feats[tokperm].transpose(2, 1, 0).reshape(2, NK))  # col = k*SN + n
    maskT = C(np.broadcast_to(mask[tokperm].T.reshape(1, NK), (H, NK)))

    pos = f(inp["pos_embed"])[0]
    cls0 = C((f(inp["cls_token"])[0, 0] + pos[0])[:, None])
    poseT = C(pos[1:][perm].T)
    pwT = C(f(inp["patch_w"]).T)
    pb = C(f(inp["patch_b"]).reshape(DC, 128).T)

    sel = np.zeros((H, D), np.float32)
    for h in range(H):
        sel[h, h * DH:(h + 1) * DH] = 1.0

    shared = dict(
        patch_wT=pwT, patch_b=pb, poseT=poseT, cls0=cls0,
        qwT=qwT, qb=qb, kwT=kwT, vwT=vwT, lwT=lwT, owT=owT, obf=obf,
        f1wT=f1wT, f1b=f1bf, f2wT=f2wT, f2b=f2bf,
        w1T=w1T, b1=b1, w2T=w2T, b2=b2,
        featsT=featsT, maskT=maskT, sel=sel,
        onesc=np.ones((128, 1), np.float32),
        epsc=np.full((4, 1), EPS, np.float32),
        onesr=np.ones((33, 128), np.float32),
        ident=np.eye(128, dtype=np.float32),
        fnw4=C(np.broadcast_to(f(inp["fnorm_w"]), (B, D))),
        fnb4=C(np.broadcast_to(f(inp["fnorm_b"]), (B, D))),
    )

    x_img = f(inp["x_img"])
    per_core = []
    for c in range(NCORES):
        xs = x_img[c * B:(c + 1) * B]
        pat = xs.reshape(B, 3, GRID, PH, GRID, PH).transpose(0, 2, 4, 1, 3, 5).reshape(B, P, 3 * PH * PH)
        pat = pat[:, perm, :]
        per_core.append(dict(patchesT=C(pat.transpose(2, 0, 1).reshape(3 * PH * PH, B * P))))
    return shared, per_core


def build_program():
    nc = bacc.Bacc("TRN2", target_bir_lowering=False, debug=False, num_devices=NCORES)
    g = lambda n, s: nc.dram_tensor(n, s, F32, kind="ExternalInput").ap()
    L = DEPTH

    patchesT = g("patchesT", [3 * PH * PH, PTOK])
    patch_wT = g("patch_wT", [3 * PH * PH, D])
    patch_b = g("patch_b", [128, DC])
    poseT = g("poseT", [D, P])
    cls0 = g("cls0", [D, 1])
    qwT = g("qwT", [L, D, D]); qb = g("qb", [L, D, 1])
    kwT = g("kwT", [L, D, D]); vwT = g("vwT", [L, D, D])
    lwT = g("lwT", [L, D, 1])
    owT = g("owT", [L, D, D]); obf = g("obf", [L, D, 1])
    f1wT = g("f1wT", [L, D, HID]); f1b = g("f1b", [L, HID, 1])
    f2wT = g("f2wT", [L, HID, D]); f2b = g("f2b", [L, D, 1])
    w1T = g("w1T", [L, 2, CPB_HID]); b1 = g("b1", [L, CPB_HID, 1])
    w2T = g("w2T", [L, CPB_HID, H]); b2 = g("b2", [L, H, 1])
    featsT = g("featsT", [2, NK]); maskT = g("maskT", [H, NK])
    sel = g("sel", [H, D])
    onesc = g("onesc", [128, 1]); onesr = g("onesr", [33, 128])
    epsc = g("epsc", [4, 1])
    ident = g("ident", [128, 128])
    fnw4 = g("fnw4", [B, D]); fnb4 = g("fnb4", [B, D])
    out = nc.dram_tensor("out", [B, D], F32, kind="ExternalOutput").ap()
    DBG = bool(int(os.environ.get("VIT_DEBUG", "0")))
    if DBG:
        dbg_x0 = nc.dram_tensor("dbg_x0", [128, DC, TOK], F32, kind="ExternalOutput").ap()
        dbg_xc = nc.dram_tensor("dbg_xc", [128, DC, TOK], F32, kind="ExternalOutput").ap()
        dbg_ctx = nc.dram_tensor("dbg_ctx", [128, DC, CTOK], F32, kind="ExternalOutput").ap()
        dbg_kT = nc.dram_tensor("dbg_kT", [128, DC, CTOK], BF16, kind="ExternalOutput").ap()
        dbg_v = nc.dram_tensor("dbg_v", [64, B, H * VW], BF16, kind="ExternalOutput").ap()
        dbg_oT = nc.dram_tensor("dbg_oT", [128, DC, TOK], F32, kind="ExternalOutput").ap()
        dbg_x1 = nc.dram_tensor("dbg_x1", [128, DC, TOK], F32, kind="ExternalOutput").ap()
        dbg_bias = nc.dram_tensor("dbg_bias", [H, NK], F32, kind="ExternalOutput").ap()

    r32 = lambda ap: ap.bitcast(F32R)
    cdma = lambda ap: ap.rearrange("(c p) o -> p c o", p=128)

    with tile.TileContext(nc) as tc, nc.allow_low_precision(reason="fp32r compute pipeline"):
        with tc.tile_pool(name="pers", bufs=1) as pers, \
             tc.tile_pool(name="wbig", bufs=2) as wbig, \
             tc.tile_pool(name="wsm", bufs=3) as wsm, \
             tc.tile_pool(name="bias", bufs=2) as biasp, \
             tc.tile_pool(name="qmt", bufs=2) as qmtp, \
             tc.tile_pool(name="ep", bufs=2) as epp, \
             tc.tile_pool(name="h1p", bufs=3) as h1p, \
             tc.tile_pool(name="t394", bufs=6) as t394, \
             tc.tile_pool(name="bt", bufs=2) as btp, \
             tc.tile_pool(name="cpbw", bufs=2) as cpbwp, \
             tc.tile_pool(name="acc6", bufs=6, space="PSUM") as acc6, \
             tc.tile_pool(name="acc2", bufs=2, space="PSUM") as acc2, \
             tc.tile_pool(name="dscr", bufs=2, space="DRAM") as dscr:

            # ---- persistent SBUF state ----
            x_t = pers.tile([128, DC, TOK], F32R)
            xc_t = pers.tile([128, DC, TOK], F32R)
            oT_t = pers.tile([128, DC, TOK], F32R)
            ctx_t = pers.tile([128, DC, CTOK], F32R)
            kT_t = pers.tile([128, DC, CTOK], BF16)
            v_t = pers.tile([64, B, H * VW], BF16)
            srec_t = pers.tile([H, TOK], F32R)
            stats_t = pers.tile([128, TOK], F32R)
            onesc_t = pers.tile([128, 1], F32R)
            eps_t = pers.tile([4, 1], F32)
            onesr_t = pers.tile([33, 128], F32R)
            sel_t = pers.tile([H, D], F32R)
            ident_t = pers.tile([128, 128], F32R)
            fnw_t = pers.tile([B, D], F32)
            fnb_t = pers.tile([B, D], F32)
            pb_t = pers.tile([128, DC], F32)
            pose_t = pers.tile([128, DC, P], F32)
            cls0_t = pers.tile([128, DC, 1], F32)
            cls_sb = pers.tile([B, D], F32)
            er_t = pers.tile([1, PTOK], F32R)
            gs_t = pers.tile([1, PTOK // 4], F32)
            gr_t = pers.tile([1, PTOK // 4], F32)
            rp_t = pers.tile([1, PTOK], F32)
            xm4 = pers.tile([B, D], F32)
            sq4 = pers.tile([B, D], F32)
            sc4 = pers.tile([B, 4], F32)   # columns: sum, mean, var, sd

            nc.sync.dma_start(onesc_t[:], r32(onesc))
            nc.sync.dma_start(eps_t[:], epsc)
            nc.sync.dma_start(onesr_t[:], r32(onesr))
            nc.sync.dma_start(sel_t[:], r32(sel))
            nc.sync.dma_start(ident_t[:], r32(ident))
            nc.sync.dma_start(fnw_t[:], fnw4)
            nc.sync.dma_start(fnb_t[:], fnb4)
            nc.sync.dma_start(pb_t[:], patch_b)
            nc.sync.dma_start(pose_t[:], poseT.rearrange("(c p) n -> p c n", p=128))
            nc.sync.dma_start(cls0_t[:], cls0.rearrange("(c p) o -> p c o", p=128))
            # ones column of v (row-sum trick), written once
            nc.vector.tensor_scalar_mul(
                v_t[:].rearrange("k b (h w) -> k b h w", w=VW)[:, :, :, DH:DH + 1],
                onesc_t[0:64, :].unsqueeze(1).unsqueeze(1).broadcast_to([64, B, H, 1]), 1.0)

            # ---- patch embed ----
            pat_t = wbig.tile([128, DC, PTOK], F32R, tag="wbig")
            nc.sync.dma_start(pat_t[:], r32(patchesT.rearrange("(c p) n -> p c n", p=128)))
            pw_t = wbig.tile([128, DC, D], F32R, tag="wbig")
            nc.sync.dma_start(pw_t[:], r32(cdma(patch_wT)))
            for mt in range(DC):
                xv = x_t[:, mt, :].rearrange("p (b t) -> p b t", t=SN)
                for hp in range(2):
                    ps = acc2.tile([128, PHALF], F32, tag="a2")
                    for kc in range(DC):
                        nc.tensor.matmul(ps[:], pw_t[:, kc, mt * 128:(mt + 1) * 128],
                                         pat_t[:, kc, hp * PHALF:(hp + 1) * PHALF],
                                         start=(kc == 0), stop=(kc == DC - 1))
                    dst = xv[:, 2 * hp:2 * hp + 2, 1:1 + P]
                    src = ps[:].rearrange("p (b t) -> p b t", t=P)
                    pose_b = pose_t[:, mt, :].unsqueeze(1).broadcast_to([128, 2, P])
                    nc.vector.scalar_tensor_tensor(dst, src, pb_t[:, mt:mt + 1], pose_b,
                                                   op0=ALU.add, op1=ALU.add)
                csrc = cls0_t[:, mt, :].unsqueeze(1).broadcast_to([128, B, 1])
                nc.vector.tensor_scalar_mul(xv[:, :, 0:1], csrc, 1.0)

            if DBG:
                nc.sync.dma_start(dbg_x0, x_t[:].bitcast(F32))

            def emit_ln(src_tile):
                for hp in range(2):
                    s1 = acc2.tile([1, HALF], F32, tag="a2")
                    for kc in range(DC):
                        nc.tensor.matmul(s1[:], onesc_t[:], src_tile[:, kc, hp * HALF:(hp + 1) * HALF],
                                         start=(kc == 0), stop=(kc == DC - 1), skip_group_check=True)
                    nc.vector.tensor_scalar_mul(stats_t[0:1, hp * HALF:(hp + 1) * HALF], s1[:], 1.0 / D)
                for hp in range(2):
                    s2 = acc2.tile([1, HALF], F32, tag="a2")
                    for kc in range(DC):
                        sq = t394.tile([128, HALF], F32R, tag="t394")
                        nc.scalar.activation(sq[:], src_tile[:, kc, hp * HALF:(hp + 1) * HALF], AF.Square)
                        nc.tensor.matmul(s2[:], onesc_t[:], sq[:],
                                         start=(kc == 0), stop=(kc == DC - 1), skip_group_check=True)
                    m = stats_t[0:1, hp * HALF:(hp + 1) * HALF]
                    m2 = stats_t[64:65, hp * HALF:(hp + 1) * HALF]
                    nc.vector.tensor_mul(m2, m, m)
                    nc.vector.scalar_tensor_tensor(m2, s2[:], 1.0 / D, m2, op0=ALU.mult, op1=ALU.subtract)
                    sd = stats_t[96:97, hp * HALF:(hp + 1) * HALF]
                    nc.scalar.activation(sd, m2, AF.Sqrt, bias=eps_t[0:1, :])
                    nc.vector.reciprocal(stats_t[32:33, hp * HALF:(hp + 1) * HALF], sd)
                for hp in range(2):
                    mB = acc6.tile([128, HALF], F32, tag="a6")
                    nc.tensor.matmul(mB[:], onesr_t[0:1, :], stats_t[0:1, hp * HALF:(hp + 1) * HALF],
                                     start=True, stop=True)
                    rB = acc6.tile([128, HALF], F32, tag="a6")
                    nc.tensor.matmul(rB[:], onesr_t[32:33, :], stats_t[32:33, hp * HALF:(hp + 1) * HALF],
                                     start=True, stop=True)
                    for kc in range(DC):
                        xm = t394.tile([128, HALF], F32, tag="t394")
                        nc.vector.tensor_sub(xm[:], src_tile[:, kc, hp * HALF:(hp + 1) * HALF], mB[:])
                        nc.vector.tensor_mul(xc_t[:, kc, hp * HALF:(hp + 1) * HALF], xm[:], rB[:])

            for l in range(DEPTH):
                # ===== CPB relative-position bias -> DRAM scratch =====
                scr = dscr.tile([H, NK], F32, tag="scr")
                w1_t = cpbwp.tile([2, CPB_HID], F32R, tag="w1")
                nc.sync.dma_start(w1_t[:], r32(w1T[l]))
                b1_t = cpbwp.tile([CPB_HID, 1], F32, tag="b1")
                nc.sync.dma_start(b1_t[:], b1[l])
                w2_t = cpbwp.tile([CPB_HID, H], F32R, tag="w2")
                nc.sync.dma_start(w2_t[:], r32(w2T[l]))
                b2_t = cpbwp.tile([H, 1], F32, tag="b2")
                nc.sync.dma_start(b2_t[:], b2[l])
                for cc in range((NK + CPC - 1) // CPC):
                    c0 = cc * CPC
                    cw = min(CPC, NK - c0)
                    fcc = t394.tile([2, CPC], F32R, tag="t394")
                    nc.sync.dma_start(fcc[:, 0:cw], r32(featsT[:, c0:c0 + cw]))
                    mcc = t394.tile([H, CPC], F32, tag="t394")
                    nc.sync.dma_start(mcc[:, 0:cw], maskT[:, c0:c0 + cw])
                    hps = acc6.tile([CPB_HID, CPC], F32, tag="a6")
                    nc.tensor.matmul(hps[:, 0:cw], w1_t[:], fcc[:, 0:cw], start=True, stop=True)
                    hcp = t394.tile([CPB_HID, CPC], F32R, tag="t394")
                    nc.scalar.activation(hcp[:, 0:cw], hps[:, 0:cw], AF.Gelu, bias=b1_t[:])
                    bps = acc6.tile([H, CPC], F32, tag="a6")
                    nc.tensor.matmul(bps[:, 0:cw], w2_t[:], hcp[:, 0:cw], start=True, stop=True)
                    bcc = t394.tile([H, CPC], F32, tag="t394")
                    nc.vector.scalar_tensor_tensor(bcc[:, 0:cw], bps[:, 0:cw], b2_t[:], mcc[:, 0:cw],
                                                   op0=ALU.add, op1=ALU.mult)
                    nc.sync.dma_start(scr[:, c0:c0 + cw], bcc[:, 0:cw])

                # ===== LN1 =====
                emit_ln(x_t)

                # ===== context pooling =====
                lw_t = biasp.tile([128, DC, 1], F32R, tag="lw")
                nc.sync.dma_start(lw_t[:], r32(cdma(lwT[l])))
                for hp in range(2):
                    scr_ps = acc2.tile([1, PHALF], F32, tag="a2")
                    for kc in range(DC):
                        rhs = xc_t[:, kc, :].rearrange("p (b t) -> p b t", t=SN)[:, 2 * hp:2 * hp + 2, 1:1 + P]
                        nc.tensor.matmul(scr_ps[:], lw_t[:, kc, :], rhs,
                                         start=(kc == 0), stop=(kc == DC - 1), skip_group_check=True)
                    nc.scalar.activation(er_t[0:1, hp * PHALF:(hp + 1) * PHALF], scr_ps[:], AF.Exp)
                er = er_t[0:1, :]
                nc.vector.reduce_sum(gs_t[:], er.rearrange("o (g s) -> o g s", s=4), axis=AX.X)
                nc.vector.reciprocal(gr_t[:], gs_t[:])
                gb = gr_t[:].unsqueeze(2).broadcast_to([1, PTOK // 4, 4])
                nc.vector.tensor_mul(er.rearrange("o (g s) -> o g s", s=4),
                                     er.rearrange("o (g s) -> o g s", s=4), gb)
                for hp in range(2):
                    wB = acc2.tile([128, PHALF], F32, tag="a2")
                    nc.tensor.matmul(wB[:], onesr_t[0:1, :], er_t[0:1, hp * PHALF:(hp + 1) * PHALF],
                                     start=True, stop=True)
                    for kc in range(DC):
                        wx = t394.tile([128, PHALF], F32, tag="t394")
                        xpat = xc_t[:, kc, :].rearrange("p (b t) -> p b t", t=SN)[:, 2 * hp:2 * hp + 2, 1:1 + P]
                        nc.vector.tensor_mul(wx[:].rearrange("p (b t) -> p b t", t=P), xpat,
                                             wB[:].rearrange("p (b t) -> p b t", t=P))
                        cdst = ctx_t[:, kc, :].rearrange("p (b c) -> p b c", c=CTXP)[:, 2 * hp:2 * hp + 2, 1:1 + T]
                        nc.vector.reduce_sum(cdst, wx[:].rearrange("p (b t s) -> p b t s", b=2, s=4), axis=AX.X)
                for kc in range(DC):
                    csrc = xc_t[:, kc, :].rearrange("p (b t) -> p b t", t=SN)[:, :, 0:1]
                    cdst = ctx_t[:, kc, :].rearrange("p (b c) -> p b c", c=CTXP)[:, :, 0:1]
                    nc.vector.tensor_scalar_mul(cdst, csrc, 1.0)

                if DBG and l == 0:
                    nc.sync.dma_start(dbg_xc, xc_t[:].bitcast(F32))
                    nc.sync.dma_start(dbg_ctx, ctx_t[:].bitcast(F32))
                    nc.sync.dma_start(dbg_bias, scr[:])
                # ===== k / v projections =====
                kw_t = wbig.tile([128, DC, D], F32R, tag="wbig")
                nc.sync.dma_start(kw_t[:], r32(cdma(kwT[l])))
                kps = [acc6.tile([128, CTOK], F32, tag="a6", name=f"kps{_i}") for _i in range(DC)]
                for kc in range(DC):
                    for mt in range(DC):
                        nc.tensor.matmul(kps[mt][:], kw_t[:, kc, mt * 128:(mt + 1) * 128], ctx_t[:, kc, :],
                                         start=(kc == 0), stop=(kc == DC - 1), skip_group_check=True)
                for mt in range(DC):
                    nc.vector.tensor_scalar_mul(kT_t[:, mt, :], kps[mt][:], 1.0)
                vw_t = wbig.tile([128, DC, D], F32R, tag="wbig")
                nc.sync.dma_start(vw_t[:], r32(cdma(vwT[l])))
                for b in range(B):
                    for hp in range(2):
                        vps = acc2.tile([64, D // 2], F32, tag="a2")
                        for kc in range(DC):
                            nc.tensor.matmul(vps[0:K, :], ctx_t[:, kc, b * CTXP:b * CTXP + K],
                                             vw_t[:, kc, hp * (D // 2):(hp + 1) * (D // 2)],
                                             start=(kc == 0), stop=(kc == DC - 1), skip_group_check=True)
                        vdst = v_t[0:K, b, hp * 6 * VW:(hp * 6 + 6) * VW].rearrange("k (h w) -> k h w", w=VW)[:, :, 0:DH]
                        nc.vector.tensor_scalar_mul(vdst, vps[0:K, :].rearrange("k (h w) -> k h w", w=DH), 1.0)

                if DBG and l == 0:
                    nc.sync.dma_start(dbg_kT, kT_t[:])
                    nc.sync.dma_start(dbg_v, v_t[:])
                # ===== attention =====
                qw_t = wbig.tile([128, DC, D], F32R, tag="wbig")
                nc.sync.dma_start(qw_t[:], r32(cdma(qwT[l])))
                qb_t = biasp.tile([128, DC], F32, tag="qb")
                nc.sync.dma_start(qb_t[:], qb[l].rearrange("(c p) o -> p (c o)", p=128))
                for mt in range(DC):
                    q_mt = qmtp.tile([128, TOK], BF16, tag="qmt")
                    for hp in range(2):
                        qps = acc2.tile([128, HALF], F32, tag="a2")
                        for kc in range(DC):
                            nc.tensor.matmul(qps[:], qw_t[:, kc, mt * 128:(mt + 1) * 128],
                                             xc_t[:, kc, hp * HALF:(hp + 1) * HALF],
                                             start=(kc == 0), stop=(kc == DC - 1), skip_group_check=True)
                        nc.vector.tensor_scalar_add(q_mt[:, hp * HALF:(hp + 1) * HALF], qps[:], qb_t[:, mt:mt + 1])
                    bT = btp.tile([64, 2, SN], F32, tag="bt")
                    for hh in range(2):
                        nc.sync.dma_start(bT[0:K, hh, :], scr[2 * mt + hh, :].rearrange("(k n) -> k n", n=SN))
                    e_mt = epp.tile([64, 2, TOK], BF16, tag="ep")
                    for hh in range(2):
                        h = 2 * mt + hh
                        off = hh * 64
                        for bp in range(2):
                            lps = acc6.tile([64, 2 * SN], F32, tag="a6")
                            for bb in range(2):
                                b = 2 * bp + bb
                                nc.tensor.matmul(lps[0:K, bb * SN:(bb + 1) * SN],
                                                 kT_t[off:off + DH, mt, b * CTXP:b * CTXP + K],
                                                 q_mt[off:off + DH, b * SN:(b + 1) * SN],
                                                 start=True, stop=True, skip_group_check=True)
                            lb = t394.tile([64, 2 * SN], F32, tag="t394")
                            bview = bT[0:K, hh, :].unsqueeze(1).broadcast_to([K, 2, SN])
                            nc.vector.tensor_add(lb[0:K, :].rearrange("k (b n) -> k b n", b=2),
                                                 lps[0:K, :].rearrange("k (b n) -> k b n", b=2), bview)
                            nc.scalar.activation(e_mt[0:K, hh, bp * 2 * SN:(bp + 1) * 2 * SN], lb[0:K, :], AF.Exp)
                        for bp in range(2):
                            ops = acc6.tile([VW, 2 * SN], F32, tag="a6")
                            for bb in range(2):
                                b = 2 * bp + bb
                                nc.tensor.matmul(ops[:, bb * SN:(bb + 1) * SN],
                                                 v_t[0:K, b, h * VW:(h + 1) * VW],
                                                 e_mt[0:K, hh, b * SN:(b + 1) * SN],
                                                 start=True, stop=True, skip_group_check=True)
                            nc.vector.tensor_scalar_mul(
                                oT_t[off:off + DH, mt, bp * 2 * SN:(bp + 1) * 2 * SN], ops[0:DH, :], 1.0)
                            rc = t394.tile([1, 2 * SN], F32R, tag="t394", name=f"rc{mt}_{hh}_{bp}")
                            nc.vector.reciprocal(rc[:], ops[DH:DH + 1, :])
                            nc.sync.dma_start(srec_t[h:h + 1, bp * 2 * SN:(bp + 1) * 2 * SN], rc[:])
                for kc in range(DC):
                    for hp in range(2):
                        rb = acc6.tile([128, HALF], F32, tag="a6")
                        nc.tensor.matmul(rb[:], sel_t[:, kc * 128:(kc + 1) * 128],
                                         srec_t[:, hp * HALF:(hp + 1) * HALF], start=True, stop=True)
                        nc.vector.tensor_mul(oT_t[:, kc, hp * HALF:(hp + 1) * HALF],
                                             oT_t[:, kc, hp * HALF:(hp + 1) * HALF], rb[:])

                if DBG and l == 0:
                    nc.sync.dma_start(dbg_oT, oT_t[:].bitcast(F32))
                # ===== out projection + residual =====
                ow_t = wbig.tile([128, DC, D], F32R, tag="wbig")
                nc.sync.dma_start(ow_t[:], r32(cdma(owT[l])))
                ob_t = biasp.tile([128, DC], F32, tag="ob")
                nc.sync.dma_start(ob_t[:], obf[l].rearrange("(c p) o -> p (c o)", p=128))
                for mt in range(DC):
                    for hp in range(2):
                        ps = acc2.tile([128, HALF], F32, tag="a2")
                        for kc in range(DC):
                            nc.tensor.matmul(ps[:], ow_t[:, kc, mt * 128:(mt + 1) * 128],
                                             oT_t[:, kc, hp * HALF:(hp + 1) * HALF],
                                             start=(kc == 0), stop=(kc == DC - 1), skip_group_check=True)
                        xs = x_t[:, mt, hp * HALF:(hp + 1) * HALF]
                        nc.vector.scalar_tensor_tensor(xs, ps[:], ob_t[:, mt:mt + 1], xs,
                                                       op0=ALU.add, op1=ALU.add)

                # ===== LN2 + MLP (fc1 -> gelu -> fc2 fused) =====
                emit_ln(x_t)
                f1b_t = biasp.tile([128, HC], F32, tag="f1b")
                nc.sync.dma_start(f1b_t[:], f1b[l].rearrange("(c p) o -> p (c o)", p=128))
                f2b_t = biasp.tile([128, DC], F32, tag="f2b")
                nc.sync.dma_start(f2b_t[:], f2b[l].rearrange("(c p) o -> p (c o)", p=128))
                for hp in range(2):
                    outps = [acc6.tile([128, HALF], F32, tag="a6", name=f"outps{hp}_{_i}") for _i in range(DC)]
                    for gg in range(4):
                        f1_t = wbig.tile([128, DC, D], F32R, tag="wbig")
                        nc.sync.dma_start(f1_t[:], r32(cdma(f1wT[l][:, gg * D:(gg + 1) * D])))
                        for hl in range(6):
                            hc = gg * 6 + hl
                            f1ps = acc2.tile([128, HALF], F32, tag="a2")
                            for kc in range(DC):
                                nc.tensor.matmul(f1ps[:], f1_t[:, kc, hl * 128:(hl + 1) * 128],
                                                 xc_t[:, kc, hp * HALF:(hp + 1) * HALF],
                                                 start=(kc == 0), stop=(kc == DC - 1), skip_group_check=True)
                            h1 = h1p.tile([128, HALF], F32R, tag="h1")
                            nc.scalar.activation(h1[:], f1ps[:], AF.Gelu, bias=f1b_t[:, hc:hc + 1])
                            f2_t = wsm.tile([128, D], F32R, tag="f2")
                            nc.sync.dma_start(f2_t[:], r32(f2wT[l][hc * 128:(hc + 1) * 128, :]))
                            for mt in range(DC):
                                nc.tensor.matmul(outps[mt][:], f2_t[:, mt * 128:(mt + 1) * 128], h1[:],
                                                 start=(hc == 0), stop=(hc == HC - 1), skip_group_check=True)
                    for mt in range(DC):
                        xs = x_t[:, mt, hp * HALF:(hp + 1) * HALF]
                        nc.vector.scalar_tensor_tensor(xs, outps[mt][:], f2b_t[:, mt:mt + 1], xs,
                                                       op0=ALU.add, op1=ALU.add)

            if DBG:
                nc.sync.dma_start(dbg_x1, x_t[:].bitcast(F32))
            # ---- final layernorm on cls tokens ----
            for kc in range(DC):
                tp = acc2.tile([B, 128], F32R, tag="a2")
                nc.tensor.transpose(tp[:], x_t[:, kc, :].rearrange("p (b t) -> p b t", t=SN)[:, :, 0], ident_t[:])
                nc.vector.tensor_scalar_mul(cls_sb[:, kc * 128:(kc + 1) * 128], tp[:], 1.0)
            nc.vector.reduce_sum(sc4[:, 0:1], cls_sb[:], axis=AX.X)
            nc.vector.tensor_scalar_mul(sc4[:, 1:2], sc4[:, 0:1], 1.0 / D)
            nc.vector.tensor_scalar_sub(xm4[:], cls_sb[:], sc4[:, 1:2])
            nc.vector.tensor_mul(sq4[:], xm4[:], xm4[:])
            nc.vector.reduce_sum(sc4[:, 2:3], sq4[:], axis=AX.X)
            nc.vector.tensor_scalar_mul(sc4[:, 2:3], sc4[:, 2:3], 1.0 / D)
            nc.scalar.activation(sc4[:, 3:4], sc4[:, 2:3], AF.Sqrt, bias=eps_t[:])
            nc.vector.reciprocal(sc4[:, 0:1], sc4[:, 3:4])
            nc.vector.tensor_scalar_mul(xm4[:], xm4[:], sc4[:, 0:1])
            nc.vector.tensor_mul(xm4[:], xm4[:], fnw_t[:])
            nc.vector.tensor_add(xm4[:], xm4[:], fnb_t[:])
            nc.sync.dma_start(out, xm4[:])

    nc.compile()
    return nc


_cached = None
last_exec_ns = None
last_result = None


def kernel(**inputs) -> np.ndarray:
    global _cached, last_exec_ns, last_result
    shared, per_core = prep_inputs(inputs)
    if _cached is None:
        _cached = build_program()
    nc = _cached
    in_maps = [dict(shared, **per_core[c]) for c in range(NCORES)]
    trace = bool(int(os.environ.get("VIT_TRACE", "0")))
    tdir = os.environ.get("VIT_TRACE_DIR") or None
    try:
        res = run_bass_kernel_spmd(nc, in_maps, core_ids=list(range(NCORES)), trace=trace,
                                   tmpdir=tdir)
    except ModuleNotFoundError:
        res = run_bass_kernel_spmd(nc, in_maps, core_ids=list(range(NCORES)), trace=False)
    last_exec_ns = res.exec_time_ns
    last_result = res
    outs = [res.results[c]["out"] for c in range(NCORES)]
    return np.concatenate(outs, axis=0)



# revision 12
# speedup vs baseline: 1.1156x; 1.1156x over previous
# ContextViT v57 Trainium2 Bass kernel.
# Strategy: data-parallel over batch (4 images per NeuronCore x 8 cores), no collectives.
# Activations live feature-major in SBUF ([768, tokens]) for the whole 12-block stack;
# weights stream from HBM. Matmuls run as fp32r (1 cyc/row at moving dim >= 256).
import os
import numpy as np

import concourse.tile as tile
from concourse import bacc, mybir
from concourse.bass_utils import run_bass_kernel_spmd

F32 = mybir.dt.float32
F32R = mybir.dt.float32r
BF16 = mybir.dt.bfloat16
AF = mybir.ActivationFunctionType
ALU = mybir.AluOpType
AX = mybir.AxisListType

DEPTH = int(os.environ.get("VIT_DEPTH", "12"))
D = 768
H = 12
DH = 64
GRID = 14
PH = 16
P = GRID * GRID            # 196 patches
SN = P + 1                 # 197 tokens per sample
TD = 2
T = P // (TD * TD)         # 49 tiles
K = 1 + T                  # 50 ctx tokens
HID = 4 * D
CPB_HID = 32
NCORES = 8
BTOT = 32
B = BTOT // NCORES         # 4 samples per core
TOK = B * SN               # 788 columns
DC = D // 128              # 6 feature chunks
HC = HID // 128            # 24 hidden chunks
HALF = TOK // 2            # 394
CTXP = 64                  # padded ctx tokens per sample
CTOK = B * CTXP            # 256
PTOK = B * P               # 784 patch cols
PHALF = PTOK // 2          # 392
VW = 65                    # v cols per head (64 + ones col)
NK = SN * K                # 9850 cpb positions
CPC = 256                  # cpb col chunk
EPS = 1e-5


def _tile_major_perm():
    perm = []
    for ti in range(GRID // TD):
        for tj in range(GRID // TD):
            for a in range(TD):
                for b in range(TD):
                    perm.append((TD * ti + a) * GRID + (TD * tj + b))
    return np.array(perm, dtype=np.int64)


def prep_inputs(inp):
    """Host-side layout prep + algebraic weight folds. Returns (shared dict, per-core list)."""
    perm = _tile_major_perm()
    f = lambda a: np.asarray(a, dtype=np.float32)
    C = np.ascontiguousarray

    n1w = f(inp["n1_w"]); n1b = f(inp["n1_b"])
    n2w = f(inp["n2_w"]); n2b = f(inp["n2_b"])
    qw = f(inp["q_w"]); kvw = f(inp["kv_w"])
    ow = f(inp["out_w"]); ob = f(inp["out_b"])
    lw = f(inp["logit_w"])
    f1w = f(inp["fc1_w"]); f1b = f(inp["fc1_b"])
    f2w = f(inp["fc2_w"]); f2b = f(inp["fc2_b"])

    L = DEPTH
    sc = 1.0 / np.sqrt(DH)
    qwT = np.stack([C((qw[l] * n1w[l][None, :] * sc).T) for l in range(L)])
    qb = np.stack([(qw[l] @ n1b[l] * sc)[:, None] for l in range(L)])
    kwT = np.stack([C((kvw[l][:D] * n1w[l][None, :]).T) for l in range(L)])
    vwT = np.stack([C((kvw[l][D:] * n1w[l][None, :]).T) for l in range(L)])
    lwT = np.stack([(lw[l][0] * n1w[l])[:, None] for l in range(L)])
    owT = np.stack([C(ow[l].T) for l in range(L)])
    obf = np.stack([(ob[l] + ow[l] @ (kvw[l][D:] @ n1b[l]))[:, None] for l in range(L)])
    f1wT = np.stack([C((f1w[l] * n2w[l][None, :]).T) for l in range(L)])
    f1bf = np.stack([(f1b[l] + f1w[l] @ n2b[l])[:, None] for l in range(L)])
    f2wT = np.stack([C(f2w[l].T) for l in range(L)])
    f2bf = np.stack([f2b[l][:, None] for l in range(L)])

    w1T = np.stack([C(f(inp["cpb_w1"])[l].T) for l in range(L)])
    b1 = np.stack([f(inp["cpb_b1"])[l][:, None] for l in range(L)])
    w2T = np.stack([C(f(inp["cpb_w2"])[l].T) for l in range(L)])
    b2 = np.stack([f(inp["cpb_b2"])[l][:, None] for l in range(L)])

    feats = f(inp["cpb_feats"])
    mask = f(inp["cpb_mask"])
    tokperm = np.concatenate([[0], perm + 1])
    featsT = C(feats[tokperm].transpose(2, 1, 0).reshape(2, NK))  # col = k*SN + n
    maskT = C(np.broadcast_to(mask[tokperm].T.reshape(1, NK), (H, NK)))

    pos = f(inp["pos_embed"])[0]
    cls0 = C((f(inp["cls_token"])[0, 0] + pos[0])[:, None])
    poseT = C(pos[1:][perm].T)
    pwT = C(f(inp["patch_w"]).T)
    pb = C(f(inp["patch_b"]).reshape(DC, 128).T)

    sel = np.zeros((H, D), np.float32)
    for h in range(H):
        sel[h, h * DH:(h + 1) * DH] = 1.0

    shared = dict(
        patch_wT=pwT, patch_b=pb, poseT=poseT, cls0=cls0,
        qwT=qwT, qb=qb, kwT=kwT, vwT=vwT, lwT=lwT, owT=owT, obf=obf,
        f1wT=f1wT, f1b=f1bf, f2wT=f2wT, f2b=f2bf,
        w1T=w1T, b1=b1, w2T=w2T, b2=b2,
        featsT=featsT, maskT=maskT, sel=sel,
        onesc=np.ones((128, 1), np.float32),
        epsc=np.full((4, 1), EPS, np.float32),
        onesr=np.ones((33, 128), np.float32),
        ident=np.eye(128, dtype=np.float32),
        fnw4=C(np.broadcast_to(f(inp["fnorm_w"]), (B, D))),
        fnb4=C(np.broadcast_to(f(inp["fnorm_b"]), (B, D))),
    )

    x_img = f(inp["x_img"])
    per_core = []
    for c in range(NCORES):
        xs = x_img[c * B:(c + 1) * B]
        pat = xs.reshape(B, 3, GRID, PH, GRID, PH).transpose(0, 2, 4, 1, 3, 5).reshape(B, P, 3 * PH * PH)
        pat = pat[:, perm, :]
        per_core.append(dict(patchesT=C(pat.transpose(2, 0, 1).reshape(3 * PH * PH, B * P))))
    return shared, per_core


def build_program():
    nc = bacc.Bacc("TRN2", target_bir_lowering=False, debug=False, num_devices=NCORES)
    g = lambda n, s: nc.dram_tensor(n, s, F32, kind="ExternalInput").ap()
    L = DEPTH

    patchesT = g("patchesT", [3 * PH * PH, PTOK])
    patch_wT = g("patch_wT", [3 * PH * PH, D])
    patch_b = g("patch_b", [128, DC])
    poseT = g("poseT", [D, P])
    cls0 = g("cls0", [D, 1])
    qwT = g("qwT", [L, D, D]); qb = g("qb", [L, D, 1])
    kwT = g("kwT", [L, D, D]); vwT = g("vwT", [L, D, D])
    lwT = g("lwT", [L, D, 1])
    owT = g("owT", [L, D, D]); obf = g("obf", [L, D, 1])
    f1wT = g("f1wT", [L, D, HID]); f1b = g("f1b", [L, HID, 1])
    f2wT = g("f2wT", [L, HID, D]); f2b = g("f2b", [L, D, 1])
    w1T = g("w1T", [L, 2, CPB_HID]); b1 = g("b1", [L, CPB_HID, 1])
    w2T = g("w2T", [L, CPB_HID, H]); b2 = g("b2", [L, H, 1])
    featsT = g("featsT", [2, NK]); maskT = g("maskT", [H, NK])
    sel = g("sel", [H, D])
    onesc = g("onesc", [128, 1]); onesr = g("onesr", [33, 128])
    epsc = g("epsc", [4, 1])
    ident = g("ident", [128, 128])
    fnw4 = g("fnw4", [B, D]); fnb4 = g("fnb4", [B, D])
    out = nc.dram_tensor("out", [B, D], F32, kind="ExternalOutput").ap()
    DBG = bool(int(os.environ.get("VIT_DEBUG", "0")))
    if DBG:
        dbg_x0 = nc.dram_tensor("dbg_x0", [128, DC, TOK], F32, kind="ExternalOutput").ap()
        dbg_xc = nc.dram_tensor("dbg_xc", [128, DC, TOK], F32, kind="ExternalOutput").ap()
        dbg_ctx = nc.dram_tensor("dbg_ctx", [128, DC, CTOK], F32, kind="ExternalOutput").ap()
        dbg_kT = nc.dram_tensor("dbg_kT", [128, DC, CTOK], BF16, kind="ExternalOutput").ap()
        dbg_v = nc.dram_tensor("dbg_v", [64, B, H * VW], BF16, kind="ExternalOutput").ap()
        dbg_oT = nc.dram_tensor("dbg_oT", [128, DC, TOK], F32, kind="ExternalOutput").ap()
        dbg_x1 = nc.dram_tensor("dbg_x1", [128, DC, TOK], F32, kind="ExternalOutput").ap()
        dbg_bias = nc.dram_tensor("dbg_bias", [H, NK], F32, kind="ExternalOutput").ap()

    r32 = lambda ap: ap.bitcast(F32R)
    cdma = lambda ap: ap.rearrange("(c p) o -> p c o", p=128)

    with tile.TileContext(nc) as tc, nc.allow_low_precision(reason="fp32r compute pipeline"):
        with tc.tile_pool(name="pers", bufs=1) as pers, \
             tc.tile_pool(name="wbig", bufs=2) as wbig, \
             tc.tile_pool(name="wsm", bufs=3) as wsm, \
             tc.tile_pool(name="bias", bufs=2) as biasp, \
             tc.tile_pool(name="qmt", bufs=2) as qmtp, \
             tc.tile_pool(name="ep", bufs=2) as epp, \
             tc.tile_pool(name="h1p", bufs=3) as h1p, \
             tc.tile_pool(name="t394", bufs=6) as t394, \
             tc.tile_pool(name="bt", bufs=2) as btp, \
             tc.tile_pool(name="cpbw", bufs=2) as cpbwp, \
             tc.tile_pool(name="acc6", bufs=6, space="PSUM") as acc6, \
             tc.tile_pool(name="acc2", bufs=2, space="PSUM") as acc2, \
             tc.tile_pool(name="dscr", bufs=2, space="DRAM") as dscr:

            # ---- persistent SBUF state ----
            x_t = pers.tile([128, DC, TOK], F32R)
            xc_t = pers.tile([128, DC, TOK], F32R)
            oT_t = pers.tile([128, DC, TOK], F32R)
            ctx_t = pers.tile([128, DC, CTOK], F32R)
            kT_t = pers.tile([128, DC, CTOK], BF16)
            v_t = pers.tile([64, B, H * VW], BF16)
            srec_t = pers.tile([H, TOK], F32R)
            sden_t = pers.tile([H, TOK], F32)
            srecf_t = pers.tile([H, TOK], F32)
            lnrec_t = pers.tile([1, TOK], F32)
            lnsd_t = pers.tile([1, TOK], F32)
            stats_t = pers.tile([128, TOK], F32R)
            onesc_t = pers.tile([128, 1], F32R)
            eps_t = pers.tile([4, 1], F32)
            onesr_t = pers.tile([33, 128], F32R)
            sel_t = pers.tile([H, D], F32R)
            ident_t = pers.tile([128, 128], F32R)
            fnw_t = pers.tile([B, D], F32)
            fnb_t = pers.tile([B, D], F32)
            pb_t = pers.tile([128, DC], F32)
            pose_t = pers.tile([128, DC, P], F32)
            cls0_t = pers.tile([128, DC, 1], F32)
            cls_sb = pers.tile([B, D], F32)
            er_t = pers.tile([1, PTOK], F32R)
            gs_t = pers.tile([1, PTOK // 4], F32)
            gr_t = pers.tile([1, PTOK // 4], F32)
            rp_t = pers.tile([1, PTOK], F32)
            xm4 = pers.tile([B, D], F32)
            sq4 = pers.tile([B, D], F32)
            sc4 = pers.tile([B, 4], F32)   # columns: sum, mean, var, sd

            nc.sync.dma_start(onesc_t[:], r32(onesc))
            nc.sync.dma_start(eps_t[:], epsc)
            nc.sync.dma_start(onesr_t[:], r32(onesr))
            nc.sync.dma_start(sel_t[:], r32(sel))
            nc.sync.dma_start(ident_t[:], r32(ident))
            nc.sync.dma_start(fnw_t[:], fnw4)
            nc.sync.dma_start(fnb_t[:], fnb4)
            nc.sync.dma_start(pb_t[:], patch_b)
            nc.sync.dma_start(pose_t[:], poseT.rearrange("(c p) n -> p c n", p=128))
            nc.sync.dma_start(cls0_t[:], cls0.rearrange("(c p) o -> p c o", p=128))
            # ones column of v (row-sum trick), written once
            nc.vector.tensor_scalar_mul(
                v_t[:].rearrange("k b (h w) -> k b h w", w=VW)[:, :, :, DH:DH + 1],
                onesc_t[0:64, :].unsqueeze(1).unsqueeze(1).broadcast_to([64, B, H, 1]), 1.0)

            # ---- patch embed ----
            pat_t = wbig.tile([128, DC, PTOK], F32R, tag="wbig")
            nc.sync.dma_start(pat_t[:], r32(patchesT.rearrange("(c p) n -> p c n", p=128)))
            pw_t = wbig.tile([128, DC, D], F32R, tag="wbig")
            nc.sync.dma_start(pw_t[:], r32(cdma(patch_wT)))
            for mt in range(DC):
                xv = x_t[:, mt, :].rearrange("p (b t) -> p b t", t=SN)
                for hp in range(2):
                    ps = acc2.tile([128, PHALF], F32, tag="a2")
                    for kc in range(DC):
                        nc.tensor.matmul(ps[:], pw_t[:, kc, mt * 128:(mt + 1) * 128],
                                         pat_t[:, kc, hp * PHALF:(hp + 1) * PHALF],
                                         start=(kc == 0), stop=(kc == DC - 1))
                    dst = xv[:, 2 * hp:2 * hp + 2, 1:1 + P]
                    src = ps[:].rearrange("p (b t) -> p b t", t=P)
                    pose_b = pose_t[:, mt, :].unsqueeze(1).broadcast_to([128, 2, P])
                    nc.vector.scalar_tensor_tensor(dst, src, pb_t[:, mt:mt + 1], pose_b,
                                                   op0=ALU.add, op1=ALU.add)
                csrc = cls0_t[:, mt, :].unsqueeze(1).broadcast_to([128, B, 1])
                nc.vector.tensor_scalar_mul(xv[:, :, 0:1], csrc, 1.0)

            if DBG:
                nc.sync.dma_start(dbg_x0, x_t[:].bitcast(F32))

            def emit_ln(src_tile):
                for hp in range(2):
                    s1 = acc2.tile([1, HALF], F32, tag="a2")
                    for kc in range(DC):
                        nc.tensor.matmul(s1[:], onesc_t[:], src_tile[:, kc, hp * HALF:(hp + 1) * HALF],
                                         start=(kc == 0), stop=(kc == DC - 1), skip_group_check=True)
                    nc.vector.tensor_scalar_mul(stats_t[0:1, hp * HALF:(hp + 1) * HALF], s1[:], 1.0 / D)
                for hp in range(2):
                    s2 = acc2.tile([1, HALF], F32, tag="a2")
                    for kc in range(DC):
                        sq = t394.tile([128, HALF], F32R, tag="t394")
                        nc.scalar.activation(sq[:], src_tile[:, kc, hp * HALF:(hp + 1) * HALF], AF.Square)
                        nc.tensor.matmul(s2[:], onesc_t[:], sq[:],
                                         start=(kc == 0), stop=(kc == DC - 1), skip_group_check=True)
                    m = stats_t[0:1, hp * HALF:(hp + 1) * HALF]
                    m2 = stats_t[64:65, hp * HALF:(hp + 1) * HALF]
                    nc.vector.tensor_mul(m2, m, m)
                    nc.vector.scalar_tensor_tensor(m2, s2[:], 1.0 / D, m2, op0=ALU.mult, op1=ALU.subtract)
                    sd = lnsd_t[0:1, hp * HALF:(hp + 1) * HALF]
                    nc.scalar.activation(sd, m2, AF.Sqrt, bias=eps_t[0:1, :])
                nc.vector.reciprocal_approx_fast(lnrec_t[:], lnsd_t[:])
                nc.vector.tensor_scalar_mul(stats_t[32:33, :], lnrec_t[:], 1.0)
                for hp in range(2):
                    mB = acc6.tile([128, HALF], F32, tag="a6")
                    nc.tensor.matmul(mB[:], onesr_t[0:1, :], stats_t[0:1, hp * HALF:(hp + 1) * HALF],
                                     start=True, stop=True)
                    rB = acc6.tile([128, HALF], F32, tag="a6")
                    nc.tensor.matmul(rB[:], onesr_t[32:33, :], stats_t[32:33, hp * HALF:(hp + 1) * HALF],
                                     start=True, stop=True)
                    for kc in range(DC):
                        xm = t394.tile([128, HALF], F32, tag="t394")
                        nc.vector.tensor_sub(xm[:], src_tile[:, kc, hp * HALF:(hp + 1) * HALF], mB[:])
                        nc.vector.tensor_mul(xc_t[:, kc, hp * HALF:(hp + 1) * HALF], xm[:], rB[:])

            for l in range(DEPTH):
                # ===== CPB relative-position bias -> DRAM scratch =====
                scr = dscr.tile([H, NK], F32, tag="scr")
                w1_t = cpbwp.tile([2, CPB_HID], F32R, tag="w1")
                nc.sync.dma_start(w1_t[:], r32(w1T[l]))
                b1_t = cpbwp.tile([CPB_HID, 1], F32, tag="b1")
                nc.sync.dma_start(b1_t[:], b1[l])
                w2_t = cpbwp.tile([CPB_HID, H], F32R, tag="w2")
                nc.sync.dma_start(w2_t[:], r32(w2T[l]))
                b2_t = cpbwp.tile([H, 1], F32, tag="b2")
                nc.sync.dma_start(b2_t[:], b2[l])
                for cc in range((NK + CPC - 1) // CPC):
                    c0 = cc * CPC
                    cw = min(CPC, NK - c0)
                    fcc = t394.tile([2, CPC], F32R, tag="t394")
                    nc.sync.dma_start(fcc[:, 0:cw], r32(featsT[:, c0:c0 + cw]))
                    mcc = t394.tile([H, CPC], F32, tag="t394")
                    nc.sync.dma_start(mcc[:, 0:cw], maskT[:, c0:c0 + cw])
                    hps = acc6.tile([CPB_HID, CPC], F32, tag="a6")
                    nc.tensor.matmul(hps[:, 0:cw], w1_t[:], fcc[:, 0:cw], start=True, stop=True)
                    hcp = t394.tile([CPB_HID, CPC], F32R, tag="t394")
                    nc.scalar.activation(hcp[:, 0:cw], hps[:, 0:cw], AF.Gelu, bias=b1_t[:])
                    bps = acc6.tile([H, CPC], F32, tag="a6")
                    nc.tensor.matmul(bps[:, 0:cw], w2_t[:], hcp[:, 0:cw], start=True, stop=True)
                    bcc = t394.tile([H, CPC], F32, tag="t394")
                    nc.vector.scalar_tensor_tensor(bcc[:, 0:cw], bps[:, 0:cw], b2_t[:], mcc[:, 0:cw],
                                                   op0=ALU.add, op1=ALU.mult)
                    nc.sync.dma_start(scr[:, c0:c0 + cw], bcc[:, 0:cw])

                # ===== LN1 =====
                emit_ln(x_t)

                # ===== context pooling =====
                lw_t = biasp.tile([128, DC, 1], F32R, tag="lw")
                nc.sync.dma_start(lw_t[:], r32(cdma(lwT[l])))
                for hp in range(2):
                    scr_ps = acc2.tile([1, PHALF], F32, tag="a2")
                    for kc in range(DC):
                        rhs = xc_t[:, kc, :].rearrange("p (b t) -> p b t", t=SN)[:, 2 * hp:2 * hp + 2, 1:1 + P]
                        nc.tensor.matmul(scr_ps[:], lw_t[:, kc, :], rhs,
                                         start=(kc == 0), stop=(kc == DC - 1), skip_group_check=True)
                    nc.scalar.activation(er_t[0:1, hp * PHALF:(hp + 1) * PHALF], scr_ps[:], AF.Exp)
                er = er_t[0:1, :]
                nc.vector.reduce_sum(gs_t[:], er.rearrange("o (g s) -> o g s", s=4), axis=AX.X)
                nc.vector.reciprocal_approx_fast(gr_t[:], gs_t[:])
                gb = gr_t[:].unsqueeze(2).broadcast_to([1, PTOK // 4, 4])
                nc.vector.tensor_mul(er.rearrange("o (g s) -> o g s", s=4),
                                     er.rearrange("o (g s) -> o g s", s=4), gb)
                for hp in range(2):
                    wB = acc2.tile([128, PHALF], F32, tag="a2")
                    nc.tensor.matmul(wB[:], onesr_t[0:1, :], er_t[0:1, hp * PHALF:(hp + 1) * PHALF],
                                     start=True, stop=True)
                    for kc in range(DC):
                        wx = t394.tile([128, PHALF], F32, tag="t394")
                        xpat = xc_t[:, kc, :].rearrange("p (b t) -> p b t", t=SN)[:, 2 * hp:2 * hp + 2, 1:1 + P]
                        nc.vector.tensor_mul(wx[:].rearrange("p (b t) -> p b t", t=P), xpat,
                                             wB[:].rearrange("p (b t) -> p b t", t=P))
                        cdst = ctx_t[:, kc, :].rearrange("p (b c) -> p b c", c=CTXP)[:, 2 * hp:2 * hp + 2, 1:1 + T]
                        nc.vector.reduce_sum(cdst, wx[:].rearrange("p (b t s) -> p b t s", b=2, s=4), axis=AX.X)
                for kc in range(DC):
                    csrc = xc_t[:, kc, :].rearrange("p (b t) -> p b t", t=SN)[:, :, 0:1]
                    cdst = ctx_t[:, kc, :].rearrange("p (b c) -> p b c", c=CTXP)[:, :, 0:1]
                    nc.vector.tensor_scalar_mul(cdst, csrc, 1.0)

                if DBG and l == 0:
                    nc.sync.dma_start(dbg_xc, xc_t[:].bitcast(F32))
                    nc.sync.dma_start(dbg_ctx, ctx_t[:].bitcast(F32))
                    nc.sync.dma_start(dbg_bias, scr[:])
                # ===== k / v projections =====
                kw_t = wbig.tile([128, DC, D], F32R, tag="wbig")
                nc.sync.dma_start(kw_t[:], r32(cdma(kwT[l])))
                kps = [acc6.tile([128, CTOK], F32, tag="a6", name=f"kps{_i}") for _i in range(DC)]
                for kc in range(DC):
                    for mt in range(DC):
                        nc.tensor.matmul(kps[mt][:], kw_t[:, kc, mt * 128:(mt + 1) * 128], ctx_t[:, kc, :],
                                         start=(kc == 0), stop=(kc == DC - 1), skip_group_check=True)
                for mt in range(DC):
                    nc.vector.tensor_scalar_mul(kT_t[:, mt, :], kps[mt][:], 1.0)
                vw_t = wbig.tile([128, DC, D], F32R, tag="wbig")
                nc.sync.dma_start(vw_t[:], r32(cdma(vwT[l])))
                for b in range(B):
                    for hp in range(2):
                        vps = acc2.tile([64, D // 2], F32, tag="a2")
                        for kc in range(DC):
                            nc.tensor.matmul(vps[0:K, :], ctx_t[:, kc, b * CTXP:b * CTXP + K],
                                             vw_t[:, kc, hp * (D // 2):(hp + 1) * (D // 2)],
                                             start=(kc == 0), stop=(kc == DC - 1), skip_group_check=True)
                        vdst = v_t[0:K, b, hp * 6 * VW:(hp * 6 + 6) * VW].rearrange("k (h w) -> k h w", w=VW)[:, :, 0:DH]
                        nc.vector.tensor_scalar_mul(vdst, vps[0:K, :].rearrange("k (h w) -> k h w", w=DH), 1.0)

                if DBG and l == 0:
                    nc.sync.dma_start(dbg_kT, kT_t[:])
                    nc.sync.dma_start(dbg_v, v_t[:])
                # ===== attention =====
                qw_t = wbig.tile([128, DC, D], F32R, tag="wbig")
                nc.sync.dma_start(qw_t[:], r32(cdma(qwT[l])))
                qb_t = biasp.tile([128, DC], F32, tag="qb")
                nc.sync.dma_start(qb_t[:], qb[l].rearrange("(c p) o -> p (c o)", p=128))
                for mt in range(DC):
                    q_mt = qmtp.tile([128, TOK], BF16, tag="qmt")
                    for hp in range(2):
                        qps = acc2.tile([128, HALF], F32, tag="a2")
                        for kc in range(DC):
                            nc.tensor.matmul(qps[:], qw_t[:, kc, mt * 128:(mt + 1) * 128],
                                             xc_t[:, kc, hp * HALF:(hp + 1) * HALF],
                                             start=(kc == 0), stop=(kc == DC - 1), skip_group_check=True)
                        nc.vector.tensor_scalar_add(q_mt[:, hp * HALF:(hp + 1) * HALF], qps[:], qb_t[:, mt:mt + 1])
                    bT = btp.tile([64, 2, SN], F32, tag="bt")
                    for hh in range(2):
                        nc.sync.dma_start(bT[0:K, hh, :], scr[2 * mt + hh, :].rearrange("(k n) -> k n", n=SN))
                    e_mt = epp.tile([64, 2, TOK], BF16, tag="ep")
                    for hh in range(2):
                        h = 2 * mt + hh
                        off = hh * 64
                        for bp in range(2):
                            lps = acc6.tile([64, 2 * SN], F32, tag="a6")
                            for bb in range(2):
                                b = 2 * bp + bb
                                nc.tensor.matmul(lps[0:K, bb * SN:(bb + 1) * SN],
                                                 kT_t[off:off + DH, mt, b * CTXP:b * CTXP + K],
                                                 q_mt[off:off + DH, b * SN:(b + 1) * SN],
                                                 start=True, stop=True, skip_group_check=True)
                            lb = t394.tile([64, 2 * SN], F32, tag="t394")
                            bview = bT[0:K, hh, :].unsqueeze(1).broadcast_to([K, 2, SN])
                            nc.vector.tensor_add(lb[0:K, :].rearrange("k (b n) -> k b n", b=2),
                                                 lps[0:K, :].rearrange("k (b n) -> k b n", b=2), bview)
                            nc.scalar.activation(e_mt[0:K, hh, bp * 2 * SN:(bp + 1) * 2 * SN], lb[0:K, :], AF.Exp)
                        for bp in range(2):
                            ops = acc6.tile([VW, 2 * SN], F32, tag="a6")
                            for bb in range(2):
                                b = 2 * bp + bb
                                nc.tensor.matmul(ops[:, bb * SN:(bb + 1) * SN],
                                                 v_t[0:K, b, h * VW:(h + 1) * VW],
                                                 e_mt[0:K, hh, b * SN:(b + 1) * SN],
                                                 start=True, stop=True, skip_group_check=True)
                            nc.vector.tensor_scalar_mul(
                                oT_t[off:off + DH, mt, bp * 2 * SN:(bp + 1) * 2 * SN], ops[0:DH, :], 1.0)
                            den = t394.tile([1, 2 * SN], F32, tag="t394", name=f"den{mt}_{hh}_{bp}")
                            nc.vector.tensor_scalar_mul(den[:], ops[DH:DH + 1, :], 1.0)
                            nc.sync.dma_start(sden_t[h:h + 1, bp * 2 * SN:(bp + 1) * 2 * SN], den[:])
                nc.vector.reciprocal_approx_fast(srecf_t[:], sden_t[:])
                nc.vector.tensor_scalar_mul(srec_t[:], srecf_t[:], 1.0)
                for kc in range(DC):
                    for hp in range(2):
                        rb = acc6.tile([128, HALF], F32, tag="a6")
                        nc.tensor.matmul(rb[:], sel_t[:, kc * 128:(kc + 1) * 128],
                                         srec_t[:, hp * HALF:(hp + 1) * HALF], start=True, stop=True)
                        nc.vector.tensor_mul(oT_t[:, kc, hp * HALF:(hp + 1) * HALF],
                                             oT_t[:, kc, hp * HALF:(hp + 1) * HALF], rb[:])

                if DBG and l == 0:
                    nc.sync.dma_start(dbg_oT, oT_t[:].bitcast(F32))
                # ===== out projection + residual =====
                ow_t = wbig.tile([128, DC, D], F32R, tag="wbig")
                nc.sync.dma_start(ow_t[:], r32(cdma(owT[l])))
                ob_t = biasp.tile([128, DC], F32, tag="ob")
                nc.sync.dma_start(ob_t[:], obf[l].rearrange("(c p) o -> p (c o)", p=128))
                for mt in range(DC):
                    for hp in range(2):
                        ps = acc2.tile([128, HALF], F32, tag="a2")
                        for kc in range(DC):
                            nc.tensor.matmul(ps[:], ow_t[:, kc, mt * 128:(mt + 1) * 128],
                                             oT_t[:, kc, hp * HALF:(hp + 1) * HALF],
                                             start=(kc == 0), stop=(kc == DC - 1), skip_group_check=True)
                        xs = x_t[:, mt, hp * HALF:(hp + 1) * HALF]
                        nc.vector.scalar_tensor_tensor(xs, ps[:], ob_t[:, mt:mt + 1], xs,
                                                       op0=ALU.add, op1=ALU.add)

                # ===== LN2 + MLP (fc1 -> gelu -> fc2 fused) =====
                emit_ln(x_t)
                f1b_t = biasp.tile([128, HC], F32, tag="f1b")
                nc.sync.dma_start(f1b_t[:], f1b[l].rearrange("(c p) o -> p (c o)", p=128))
                f2b_t = biasp.tile([128, DC], F32, tag="f2b")
                nc.sync.dma_start(f2b_t[:], f2b[l].rearrange("(c p) o -> p (c o)", p=128))
                for hp in range(2):
                    outps = [acc6.tile([128, HALF], F32, tag="a6", name=f"outps{hp}_{_i}") for _i in range(DC)]
                    for gg in range(4):
                        f1_t = wbig.tile([128, DC, D], F32R, tag="wbig")
                        nc.sync.dma_start(f1_t[:], r32(cdma(f1wT[l][:, gg * D:(gg + 1) * D])))
                        for hl in range(6):
                            hc = gg * 6 + hl
                            f1ps = acc2.tile([128, HALF], F32, tag="a2")
                            for kc in range(DC):
                                nc.tensor.matmul(f1ps[:], f1_t[:, kc, hl * 128:(hl + 1) * 128],
                                                 xc_t[:, kc, hp * HALF:(hp + 1) * HALF],
                                                 start=(kc == 0), stop=(kc == DC - 1), skip_group_check=True)
                            h1 = h1p.tile([128, HALF], F32R, tag="h1")
                            nc.scalar.activation(h1[:], f1ps[:], AF.Gelu, bias=f1b_t[:, hc:hc + 1])
                            f2_t = wsm.tile([128, D], F32R, tag="f2")
                            nc.sync.dma_start(f2_t[:], r32(f2wT[l][hc * 128:(hc + 1) * 128, :]))
                            for mt in range(DC):
                                nc.tensor.matmul(outps[mt][:], f2_t[:, mt * 128:(mt + 1) * 128], h1[:],
                                                 start=(hc == 0), stop=(hc == HC - 1), skip_group_check=True)
                    for mt in range(DC):
                        xs = x_t[:, mt, hp * HALF:(hp + 1) * HALF]
                        nc.vector.scalar_tensor_tensor(xs, outps[mt][:], f2b_t[:, mt:mt + 1], xs,
                                                       op0=ALU.add, op1=ALU.add)

            if DBG:
                nc.sync.dma_start(dbg_x1, x_t[:].bitcast(F32))
            # ---- final layernorm on cls tokens ----
            for kc in range(DC):
                tp = acc2.tile([B, 128], F32R, tag="a2")
                nc.tensor.transpose(tp[:], x_t[:, kc, :].rearrange("p (b t) -> p b t", t=SN)[:, :, 0], ident_t[:])
                nc.vector.tensor_scalar_mul(cls_sb[:, kc * 128:(kc + 1) * 128], tp[:], 1.0)
            nc.vector.reduce_sum(sc4[:, 0:1], cls_sb[:], axis=AX.X)
            nc.vector.tensor_scalar_mul(sc4[:, 1:2], sc4[:, 0:1], 1.0 / D)
            nc.vector.tensor_scalar_sub(xm4[:], cls_sb[:], sc4[:, 1:2])
            nc.vector.tensor_mul(sq4[:], xm4[:], xm4[:])
            nc.vector.reduce_sum(sc4[:, 2:3], sq4[:], axis=AX.X)
            nc.vector.tensor_scalar_mul(sc4[:, 2:3], sc4[:, 2:3], 1.0 / D)
            nc.scalar.activation(sc4[:, 3:4], sc4[:, 2:3], AF.Sqrt, bias=eps_t[:])
            nc.vector.reciprocal(sc4[:, 0:1], sc4[:, 3:4])
            nc.vector.tensor_scalar_mul(xm4[:], xm4[:], sc4[:, 0:1])
            nc.vector.tensor_mul(xm4[:], xm4[:], fnw_t[:])
            nc.vector.tensor_add(xm4[:], xm4[:], fnb_t[:])
            nc.sync.dma_start(out, xm4[:])

    nc.compile()
    return nc


_cached = None
last_exec_ns = None
last_result = None


def kernel(**inputs) -> np.ndarray:
    global _cached, last_exec_ns, last_result
    shared, per_core = prep_inputs(inputs)
    if _cached is None:
        _cached = build_program()
    nc = _cached
    in_maps = [dict(shared, **per_core[c]) for c in range(NCORES)]
    trace = bool(int(os.environ.get("VIT_TRACE", "0")))
    tdir = os.environ.get("VIT_TRACE_DIR") or None
    try:
        res = run_bass_kernel_spmd(nc, in_maps, core_ids=list(range(NCORES)), trace=trace,
                                   tmpdir=tdir)
    except ModuleNotFoundError:
        res = run_bass_kernel_spmd(nc, in_maps, core_ids=list(range(NCORES)), trace=False)
    last_exec_ns = res.exec_time_ns
    last_result = res
    outs = [res.results[c]["out"] for c in range(NCORES)]
    return np.concatenate(outs, axis=0)

